# revision 10
# baseline (speedup 1.0000x reference)
"""Trainium2 Bass kernel for the adaptive semantic-scal loss (segment_reduce).

Self-contained: hardcodes shapes/sharding for
  pred [2,17,200,200,16] f32, ssc_target [2,200,200,16] int, f1_list [17] f32.

Strategy (8 NeuronCores, data-parallel over voxels):
  - host re-lays pred out voxel-blocked / class-major: [core][128][17][1250]
    so DMA per partition is contiguous and all engines run on 128 partitions
  - per core: ACT exp -> DVE class-tree-sum -> reciprocal -> per-class fused
    multiply+reduce (sum_p, nominator) and is_equal+reduce (sum_comp)
  - PE matmul collapses partitions; AllReduce(51 f32) across cores;
    the scalar loss epilogue runs on-device (identically on all cores)
"""

import sys

for _p in ("/opt/trn_rl_repo",):
    if _p not in sys.path:
        sys.path.append(_p)

import numpy as np
import ml_dtypes

import concourse.bacc as bacc
import concourse.tile as tile
import concourse.mybir as mybir
from concourse.bass_utils import run_bass_kernel_spmd

F32 = mybir.dt.float32
BF16 = mybir.dt.bfloat16
ALU = mybir.AluOpType
ACTF = mybir.ActivationFunctionType

N_CORES = 8
P = 128          # partitions
C = 17           # classes
KV = 1250        # voxels per partition per core (128*1250*8 = 1.28M)
T = 2            # tiles along voxel axis
KT = KV // T

BETA = 0.95
ALPHA = 5.0
WPC = 3.0
NEG_BIG = -1.0e30


def _build():
    nc = bacc.Bacc("TRN2", target_bir_lowering=False, debug=False,
                   num_devices=N_CORES)
    pred_d = nc.dram_tensor("pred", [P, C, KV], F32, kind="ExternalInput")
    tgt_d = nc.dram_tensor("tgt", [P, KV], BF16, kind="ExternalInput")
    f1_d = nc.dram_tensor("f1", [1, C], F32, kind="ExternalInput")
    out_d = nc.dram_tensor("out", [1, 1], F32, kind="ExternalOutput")

    with tile.TileContext(nc) as tc:
        with (
            tc.tile_pool(name="pred", bufs=2) as pk,
            tc.tile_pool(name="work", bufs=2) as pw,
            tc.tile_pool(name="small", bufs=3) as ps,
            tc.tile_pool(name="persist", bufs=1) as pa,
            tc.tile_pool(name="psum", bufs=1, space="PSUM") as pp,
            tc.tile_pool(name="dram", bufs=1, space="DRAM") as pd,
        ):
            tgt_sb = pa.tile([P, KV], BF16)
            nc.sync.dma_start(out=tgt_sb[:, :], in_=tgt_d[:, :])
            f1_sb = pa.tile([1, C], F32)
            nc.sync.dma_start(out=f1_sb[:, :], in_=f1_d[:, :])

            # accumulators: per-class (x per-tile) partial sums, fp32
            spA = pa.tile([P, C, T], F32)
            nomA = pa.tile([P, C, T], F32)
            # allP = [sum_p | nominator | sum_comp] per partition
            allP = pa.tile([P, 51], F32)
            ones_col = pa.tile([P, 1], F32)
            nc.vector.memset(ones_col[:, :], 1.0)

            # sum_comp: histogram of tgt (independent of pred)
            hdump = pa.tile([P, KV], BF16)
            for c in range(C):
                nc.vector.tensor_scalar(
                    hdump[:, :], tgt_sb[:, :], float(c), None, ALU.is_equal,
                    ALU.add, accum_out=allP[:, 34 + c:35 + c])

            for t in range(T):
                pred_t = pk.tile([P, C, KT], F32)
                nc.sync.dma_start(out=pred_t[:, :, :],
                                  in_=pred_d[:, :, t * KT:(t + 1) * KT])
                E = pw.tile([P, C, KT], BF16)
                nc.scalar.activation(E[:, :, :], pred_t[:, :, :], ACTF.Exp)

                # softmax denominator: in-place tree sum over the 17 classes
                tr = ps.tile([P, 8, KT], BF16, bufs=2)
                nc.vector.tensor_add(tr[:, :, :], E[:, 0:8, :], E[:, 8:16, :])
                nc.vector.tensor_add(tr[:, 0:4, :], tr[:, 0:4, :], tr[:, 4:8, :])
                nc.vector.tensor_add(tr[:, 0:2, :], tr[:, 0:2, :], tr[:, 2:4, :])
                nc.vector.tensor_add(tr[:, 0, :], tr[:, 0, :], tr[:, 1, :])
                S = ps.tile([P, KT], F32, bufs=2)
                nc.vector.tensor_add(S[:, :], tr[:, 0, :], E[:, 16, :])
                invf = ps.tile([P, KT], F32, bufs=2)
                nc.vector.reciprocal(invf[:, :], S[:, :])
                inv = ps.tile([P, KT], BF16, bufs=2)
                nc.vector.tensor_copy(inv[:, :], invf[:, :])

                tgt_t = tgt_sb[:, t * KT:(t + 1) * KT]
                for c in range(C):
                    R = ps.tile([P, KT], BF16)
                    # R = E_c / S ; accum = sum_k R  (-> sum_p partial)
                    nc.vector.scalar_tensor_tensor(
                        out=R[:, :], in0=E[:, c, :], scalar=0.0,
                        in1=inv[:, :], op0=ALU.add, op1=ALU.mult,
                        accum_out=spA[:, c, t:t + 1])
                    dump = ps.tile([P, KT], BF16, bufs=2)
                    # dump = (tgt == c) * R ; accum = sum_k (-> nominator)
                    nc.vector.scalar_tensor_tensor(
                        out=dump[:, :], in0=tgt_t, scalar=float(c),
                        in1=R[:, :], op0=ALU.is_equal, op1=ALU.mult,
                        accum_out=nomA[:, c, t:t + 1])

            # collapse per-tile partials
            nc.vector.tensor_reduce(allP[:, 0:17], spA[:, :, :],
                                    axis=mybir.AxisListType.X, op=ALU.add)
            nc.vector.tensor_reduce(allP[:, 17:34], nomA[:, :, :],
                                    axis=mybir.AxisListType.X, op=ALU.add)

            # collapse 128 partitions on the PE: out[51,1] = allP.T @ ones
            psum_t = pp.tile([64, 1], F32)
            nc.tensor.matmul(psum_t[0:51, :], allP[:, :], ones_col[:, :],
                             start=True, stop=True)

            sb_part = pa.tile([64, 1], F32)
            nc.vector.memset(sb_part[:, :], 0.0)
            nc.vector.tensor_copy(sb_part[0:51, :], psum_t[0:51, :])
            cc_in = pd.tile([1, 64], F32)
            cc_out = pd.tile([1, 64], F32)
            nc.sync.dma_start(out=cc_in[0, :], in_=sb_part[:, 0])
            nc.gpsimd.collective_compute(
                "AllReduce", ALU.add,
                replica_groups=[list(range(N_CORES))],
                ins=[cc_in[:, :].opt()],
                outs=[cc_out[:, :].opt()],
            )
            ep = pa.tile([1, 64], F32)
            nc.sync.dma_start(out=ep[:, :], in_=cc_out[:, :])

            # ---------------- epilogue (identical on every core) ----------
            _tn = [0]

            def tile17():
                _tn[0] += 1
                return ps.tile([1, C], F32, name="ep17_%d" % _tn[0], tag="ep17_%d" % _tn[0])

            def tile1():
                _tn[0] += 1
                return ps.tile([1, 1], F32, name="ep1_%d" % _tn[0], tag="ep1_%d" % _tn[0])

            sp = ep[:, 0:17]
            nom = ep[:, 17:34]
            ct = ep[:, 34:51]

            nmask = tile1()
            nc.vector.tensor_reduce(nmask[:, :], ct,
                                    axis=mybir.AxisListType.X, op=ALU.add)
            has = tile17()
            nc.vector.tensor_scalar(has[:, :], ct, 0.0, None, ALU.is_gt)
            pm = tile17()
            nc.vector.tensor_scalar(pm[:, :], sp, 0.0, None, ALU.is_gt)

            def guarded_div(num_ap, den_ap, gate):
                # gate * num / (den + (1-gate)) ; den >= 0, gate in {0,1}
                omg = tile17()
                nc.vector.tensor_scalar(omg[:, :], gate, -1.0, 1.0,
                                        ALU.mult, ALU.add)
                den = tile17()
                nc.vector.tensor_add(den[:, :], den_ap, omg[:, :])
                rden = tile17()
                nc.vector.reciprocal(rden[:, :], den[:, :])
                q = tile17()
                nc.vector.tensor_mul(q[:, :], num_ap, rden[:, :])
                nc.vector.tensor_mul(q[:, :], q[:, :], gate)
                return q

            prec = guarded_div(nom, sp, pm[:, :])
            rec = guarded_div(nom, ct, has[:, :])

            # neg_comp = n_mask - ct ; spec_num = (n_mask - sp) - (ct - nom)
            neg = tile17()
            nc.vector.tensor_scalar(neg[:, :], ct, nmask[:, :], -1.0,
                                    ALU.subtract, ALU.mult)
            a = tile17()
            nc.vector.tensor_scalar(a[:, :], sp, nmask[:, :], -1.0,
                                    ALU.subtract, ALU.mult)
            b = tile17()
            nc.vector.tensor_sub(b[:, :], ct, nom)
            snum = tile17()
            nc.vector.tensor_sub(snum[:, :], a[:, :], b[:, :])
            nmp = tile17()
            nc.vector.tensor_scalar(nmp[:, :], neg[:, :], 0.0, None, ALU.is_gt)
            spec = guarded_div(snum[:, :], neg[:, :], nmp[:, :])

            def bce(x):
                # min(-ln(max(x,1e-38)), 100)
                xm = tile17()
                nc.vector.tensor_scalar(xm[:, :], x, 1e-38, None, ALU.max)
                l = tile17()
                nc.scalar.activation(l[:, :], xm[:, :], ACTF.Ln)
                nl = tile17()
                nc.vector.tensor_scalar(nl[:, :], l[:, :], -1.0, 100.0,
                                        ALU.mult, ALU.min)
                return nl

            bp = bce(prec[:, :])
            br = bce(rec[:, :])
            bs = bce(spec[:, :])
            ll = tile17()
            nc.vector.tensor_mul(ll[:, :], bp[:, :], pm[:, :])
            t5 = tile17()
            nc.vector.tensor_mul(t5[:, :], bs[:, :], nmp[:, :])
            nc.vector.tensor_add(ll[:, :], ll[:, :], br[:, :])
            nc.vector.tensor_add(ll[:, :], ll[:, :], t5[:, :])
            nc.vector.tensor_mul(ll[:, :], ll[:, :], has[:, :])

            # f1 and running buffer
            dnm = tile17()
            nc.vector.tensor_add(dnm[:, :], prec[:, :], rec[:, :])
            dpos = tile17()
            nc.vector.tensor_scalar(dpos[:, :], dnm[:, :], 0.0, None, ALU.is_gt)
            f1 = guarded_div(prec[:, :], dnm[:, :], dpos[:, :])  # prec/dnm*dpos
            nc.vector.tensor_mul(f1[:, :], f1[:, :], rec[:, :])
            nc.vector.tensor_scalar(f1[:, :], f1[:, :], 2.0, None, ALU.mult)
            nc.vector.tensor_mul(f1[:, :], f1[:, :], has[:, :])  # cur_f1
            nf = tile17()
            nc.vector.tensor_scalar(nf[:, :], f1_sb[:, :], BETA, None, ALU.mult)
            nc.vector.scalar_tensor_tensor(
                out=nf[:, :], in0=f1[:, :], scalar=1.0 - BETA, in1=nf[:, :],
                op0=ALU.mult, op1=ALU.add)

            cnt = tile1()
            nc.vector.tensor_reduce(cnt[:, :], has[:, :],
                                    axis=mybir.AxisListType.X, op=ALU.add)

            # weights: softmax over selected classes
            sel = tile17()
            nc.vector.tensor_scalar(sel[:, :], ll[:, :], 0.0, None,
                                    ALU.is_equal)
            nc.vector.tensor_scalar(sel[:, :], sel[:, :], -1.0, 1.0,
                                    ALU.mult, ALU.add)  # sel = (ll != 0)
            lgs = tile17()
            nc.vector.tensor_scalar(lgs[:, :], nf[:, :], -ALPHA, ALPHA,
                                    ALU.mult, ALU.add)  # 5*(1-new_f1)
            nc.vector.tensor_mul(lgs[:, :], lgs[:, :], sel[:, :])
            toff = tile17()
            nc.vector.tensor_scalar(toff[:, :], sel[:, :], -NEG_BIG, NEG_BIG,
                                    ALU.mult, ALU.add)  # 0 if sel else -1e30
            nc.vector.tensor_add(lgs[:, :], lgs[:, :], toff[:, :])

            mx = tile1()
            nc.vector.tensor_reduce(mx[:, :], lgs[:, :],
                                    axis=mybir.AxisListType.X, op=ALU.max)
            ngm = tile1()
            nc.vector.tensor_scalar(ngm[:, :], mx[:, :], -1.0, None, ALU.mult)
            ex = tile17()
            nc.scalar.activation(ex[:, :], lgs[:, :], ACTF.Exp,
                                 bias=ngm[:, :], scale=1.0)
            se = tile1()
            nc.vector.tensor_reduce(se[:, :], ex[:, :],
                                    axis=mybir.AxisListType.X, op=ALU.add)
            rse = tile1()
            nc.vector.reciprocal(rse[:, :], se[:, :])
            sm = tile17()
            nc.vector.tensor_scalar(sm[:, :], ex[:, :], rse[:, :], None,
                                    ALU.mult)

            wp = tile1()
            nc.vector.tensor_scalar(wp[:, :], cnt[:, :], WPC, None, ALU.mult)
            wsm = tile17()
            nc.vector.tensor_scalar(wsm[:, :], sm[:, :], wp[:, :], 1.0,
                                    ALU.mult, ALU.add)
            wtd = tile17()
            nc.vector.tensor_mul(wtd[:, :], ll[:, :], wsm[:, :])
            lsum = tile1()
            nc.vector.tensor_reduce(lsum[:, :], wtd[:, :],
                                    axis=mybir.AxisListType.X, op=ALU.add)
            cd = tile1()
            nc.vector.tensor_scalar(cd[:, :], cnt[:, :], 1.0 + WPC, None,
                                    ALU.mult)
            rcd = tile1()
            nc.vector.reciprocal(rcd[:, :], cd[:, :])
            loss = tile1()
            nc.vector.tensor_mul(loss[:, :], lsum[:, :], rcd[:, :])
            nc.sync.dma_start(out=out_d[:, :], in_=loss[:, :])

    nc.compile()
    return nc


_NC_CACHE = None


def _get_nc():
    global _NC_CACHE
    if _NC_CACHE is None:
        _NC_CACHE = _build()
    return _NC_CACHE


def _shard_inputs(pred, ssc_target, f1_list):
    pred = np.asarray(pred, dtype=np.float32)
    tgt = np.asarray(ssc_target)
    f1 = np.asarray(f1_list, dtype=np.float32).reshape(1, C)

    nvox = N_CORES * P * KV
    # voxel-major [v, c], then block: [core, p, k, c] -> [core, p, c, k]
    pv = np.ascontiguousarray(
        pred.reshape(2, C, -1).transpose(0, 2, 1).reshape(nvox, C)
        .reshape(N_CORES, P, KV, C).transpose(0, 1, 3, 2))
    tv = tgt.reshape(nvox).reshape(N_CORES, P, KV).astype(np.float32).astype(
        ml_dtypes.bfloat16)
    in_maps = []
    for i in range(N_CORES):
        in_maps.append({"pred": pv[i], "tgt": tv[i], "f1": f1})
    return in_maps


def kernel(pred, ssc_target, f1_list):
    nc = _get_nc()
    in_maps = _shard_inputs(pred, ssc_target, f1_list)
    res = run_bass_kernel_spmd(nc, in_maps, core_ids=list(range(N_CORES)))
    out = np.asarray(res.results[0]["out"], dtype=np.float32)
    return out.reshape(())


if __name__ == "__main__":
    rng = np.random.default_rng(0)
    pred = rng.standard_normal((2, C, 200, 200, 16), dtype=np.float32)
    tgt = rng.integers(0, C, size=(2, 200, 200, 16)).astype(np.int64)
    f1l = np.zeros((C,), np.float32)
    print(kernel(pred, tgt, f1l))
